# revision 47
# baseline (speedup 1.0000x reference)
"""LoRA Multihead Attention on 8 TRN2 NeuronCores.

Sharding: tensor-parallel attention over heads, token-parallel epilogue.
Core c owns heads {2c, 2c+1} (= channel slice [128c, 128c+128)) for the
projections + attention, and owns tokens l in [256c, 256c+256) of each
batch for the out_proj/LoRA epilogue. Each core:
  1. computes q,k (feature-major) and v (token-major) projections for its
     heads,
  2. runs attention S^T = k^T q (row-tiled: both heads concurrently in the
     PE array), P = exp(S^T) in 1024-wide Act instructions, P@V per head
     into its own PSUM bank with 64 ones-columns appended to the v
     stationary so the softmax denominator accumulates (replicated) into
     PSUM partitions 64..127 — no separate ones-matmuls and no
     cross-partition broadcast in the normalization,
  3. AllToAll per batch: cores exchange 256-token x 128-channel blocks so
     every core ends with all 1024 channels of its own 256-token slice
     (8x less traffic than the AllGather of the full activation),
  4. computes the FULL out_proj + LoRA for its 512 tokens (both batches).
Host reassembles the 8 token slices and restores (L, N, E) layout.

Schedule: the batch-1 projections are split so each piece lands where an
engine is otherwise idle — k blocks + first q/v as fillers inside the
PE-bound batch-0 attention, the remaining q/v blocks just-in-time inside
the Act(exp)-bound batch-1 attention.  q/k bias adds run on the Act
engine during the batch-0 phase (Act idle there) and on DVE during
batch-1 (Act saturated).  The batch-0 epilogue's LoRA stationaries are
staged through a tile that (x0) depends on batch-1 lb3 output: the list
scheduler mis-times collectives and would otherwise hoist the
epilogue's Ldweights ahead of the batch-0 AllToAll (locking the PE
array), and the gate pins the whole batch-0 epilogue into the exposed
batch-1 AllToAll window at the tail, where PE idle time is wall-clock,
instead of batch-1's harmless Act-bound bubbles.  The tail's a2a_out
fetch splits its dma_start issues across SP and Act (both idle there).
Constants are host-interleaved so each
group loads as a single DMA (the per-DMA sequencer issue cost dominates
the startup otherwise), and the first q/k token block lands first.

All matmuls bf16 with fp32 PSUM accumulation; softmax statistics in
fp32.  The output is written back in bf16 (halves the final DMA
traffic; ~0.1% added noise vs the 2e-2 gate) and upcast on the host.
"""

import os
import sys
from collections import deque

sys.path.insert(0, "/opt/trn_rl_repo")

import numpy as np
import ml_dtypes

import concourse.bass as bass  # noqa: F401  (import keeps bass registered)
import concourse.tile as tile
from concourse import bacc, mybir
from concourse.bass_utils import run_bass_kernel_spmd

BF = ml_dtypes.bfloat16
bf16 = mybir.dt.bfloat16
f32 = mybir.dt.float32

L, N, E = 2048, 2, 1024
T = N * L            # 4096 tokens, t = n*L + l
H, D, R = 16, 64, 16
NCORES = 8
HPC = H // NCORES    # heads per core = 2
CS = HPC * D         # channel slice width per core = 128
TPC = T // NCORES    # tokens per core in the epilogue = 512
LPC = L // NCORES    # l-slice per core per batch = 256
SCALE = D ** -0.5
LORA_SCALING = 32.0 / 16.0

LB = 512             # l-block (moving free dim)
NT = T // LB         # 8 t-blocks over all tokens
NTB = NT // N        # 4 t-blocks per batch
NLB = L // LB        # 4 l-blocks per batch
NMT = L // 128       # 16 m-tiles per batch
NJ = NMT // 2        # 8 m-tile pairs per batch
NE = E // 128        # 8 contraction tiles
VW = 2 * D           # v stationary width per m-tile: 64 data + 64 ones cols
                     # (the ones cols replicate the softmax denominator into
                     # PSUM partitions 64..127, so normalization needs no
                     # cross-partition broadcast)

_CACHE = {}

K_GPSHIP = bool(int(os.environ.get("K_GPSHIP", "0")))


def _build_nc(reps=1):
    nc = bacc.Bacc("TRN2", target_bir_lowering=False, debug=False,
                   enable_asserts=False, num_devices=NCORES)

    qT_d = nc.dram_tensor("qT", [E, T], bf16, kind="ExternalInput")
    wqkt_d = nc.dram_tensor("wqkt", [128, NE * 2 * CS], bf16, kind="ExternalInput")
    wvt_d = nc.dram_tensor("wvt", [128, NE * CS], bf16, kind="ExternalInput")
    bqk_d = nc.dram_tensor("bqk", [128, 2], f32, kind="ExternalInput")
    woutt_d = nc.dram_tensor("woutt", [128, NE * E], bf16, kind="ExternalInput")
    at_d = nc.dram_tensor("at", [128, NE * R], bf16, kind="ExternalInput")
    btf_d = nc.dram_tensor("btf", [R, E], bf16, kind="ExternalInput")
    bout_d = nc.dram_tensor("bout", [128, NE], f32, kind="ExternalInput")
    outp_d = nc.dram_tensor("outp", [E, TPC], bf16, kind="ExternalOutput")

    a2a_in = [nc.dram_tensor(f"a2a_in{n}", [E, LPC], bf16) for n in range(N)]
    a2a_out = [nc.dram_tensor(f"a2a_out{n}", [E, LPC], bf16) for n in range(N)]

    with tile.TileContext(nc) as tc:
        with (
            tc.tile_pool(name="const", bufs=1) as cp,
            tc.tile_pool(name="qt", bufs=1) as qtp,
            tc.tile_pool(name="qks", bufs=1) as qksp,
            tc.tile_pool(name="vp", bufs=1) as vp,
            tc.tile_pool(name="pp", bufs=6) as pp,
            tc.tile_pool(name="osb", bufs=1) as osbp,
            tc.tile_pool(name="ot", bufs=2) as otp,
            tc.tile_pool(name="small", bufs=4) as smp,
            tc.tile_pool(name="ob", bufs=3) as obp,
            tc.tile_pool(name="ps_s", bufs=2, space="PSUM") as ps_s,
            tc.tile_pool(name="ps_o", bufs=1, space="PSUM") as ps_o,
            tc.tile_pool(name="ps_m", bufs=2, space="PSUM") as ps_m,
        ):
            # ---- constants & inputs (e-interleaved on the host so each
            # ---- group is a single DMA); first q/k block lands first ----
            wqkt_all = cp.tile([128, NE * 2 * CS], bf16, tag="wqkt", name="wqkt")
            wvt_all = cp.tile([128, NE * CS], bf16, tag="wvt", name="wvt")
            woutt_all = cp.tile([128, NE * E], bf16, tag="woutt", name="woutt")
            at_all = cp.tile([128, NE * R], bf16, tag="at", name="at")
            btf = cp.tile([R, E], bf16, tag="btf", name="btf")
            bqk_all = cp.tile([128, 2], f32, tag="bqk", name="bqk")
            bout_all = cp.tile([128, NE], f32, tag="bout", name="bout")
            wqkt = [wqkt_all[:, e * 2 * CS:(e + 1) * 2 * CS] for e in range(NE)]
            wvt = [wvt_all[:, e * CS:(e + 1) * CS] for e in range(NE)]
            woutt = [woutt_all[:, e * E:(e + 1) * E] for e in range(NE)]
            at = [at_all[:, e * R:(e + 1) * R] for e in range(NE)]
            bqk = [bqk_all[:, ch:ch + 1] for ch in range(2)]
            bout = [bout_all[:, e:e + 1] for e in range(NE)]
            # pre-load the exp spline tables while input DMAs run
            warm = cp.tile([1, 8], f32, tag="warm", name="warm")
            nc.vector.memset(warm[:], 0.0)
            nc.scalar.activation(warm[:], warm[:], mybir.ActivationFunctionType.Exp)

            qt = [qtp.tile([128, T], bf16, tag=f"qt{e}", name=f"qt{e}") for e in range(NE)]
            # first two e-blocks of the q/k weights land before the qt bulk so
            # the first projection matmuls can start ~1.5us earlier
            nc.sync.dma_start(wqkt_all[:, 0:4 * CS], wqkt_d.ap()[:, 0:4 * CS])
            nc.sync.dma_start(qt[0][:, 0:LB], qT_d.ap()[0:128, 0:LB])
            nc.sync.dma_start(qt[1][:, 0:LB], qT_d.ap()[128:256, 0:LB])
            nc.sync.dma_start(wqkt_all[:, 4 * CS:], wqkt_d.ap()[:, 4 * CS:])
            for e in range(2, NE):
                sl = slice(e * 128, (e + 1) * 128)
                nc.sync.dma_start(qt[e][:, 0:LB], qT_d.ap()[sl, 0:LB])
            nc.sync.dma_start(wvt_all[:], wvt_d.ap())
            nc.sync.dma_start(bqk_all[:], bqk_d.ap())
            nc.sync.dma_start(at_all[:], at_d.ap())
            nc.sync.dma_start(bout_all[:], bout_d.ap())
            nc.sync.dma_start(btf[:], btf_d.ap())
            for e in range(NE):
                sl = slice(e * 128, (e + 1) * 128)
                nc.sync.dma_start(qt[e][:, LB:L], qT_d.ap()[sl, LB:L])
            for e in range(NE):
                sl = slice(e * 128, (e + 1) * 128)
                nc.sync.dma_start(qt[e][:, L:T], qT_d.ap()[sl, L:T])
            nc.sync.dma_start(woutt_all[:], woutt_d.ap())

            for _rep in range(reps):
              qks = [qksp.tile([128, T], bf16, tag=f"qks{ch}", name=f"qks{ch}") for ch in range(2)]
              # v stationary: per (batch, head) [128, NMT*VW]; col D of each
              # m-tile block stays at the memset 1.0 -> free denominator row.
              v_all = [[vp.tile([128, NMT * VW], bf16, tag=f"v{n}{h}", name=f"v{n}{h}")
                        for h in range(2)] for n in range(N)]
              for n in range(N):
                  for h in range(2):
                      nc.vector.memset(v_all[n][h][:], 1.0)
              osb = [osbp.tile([CS, L], bf16, tag=f"osb{n}", name=f"osb{n}")
                     for n in range(N)]
              at_s = smp.tile([128, NE * R], bf16, tag="ats", name="ats")
              at_gated = [at_s[:, e * R:(e + 1) * R] for e in range(NE)]
              at_plain = at

              # ---- emission units ----
              def emit_qk(n, t, chs=(0, 1), bias_eng="act"):
                  """q (ch 0) / k (ch 1) projection for token block t of batch n.

                  bias_eng: "act" during the PE-bound batch-0 phase (Act is
                  idle there), "dve" during the Act-bound batch-1 phase.
                  """
                  tb = n * NTB + t
                  cs = slice(tb * LB, (tb + 1) * LB)
                  for ch in chs:
                      pm = ps_m.tile([128, LB], f32, tag="m", name="pm")
                      for e in range(NE):
                          nc.tensor.matmul(pm[:], wqkt_all[:, e * 2 * CS + ch * CS:e * 2 * CS + (ch + 1) * CS],
                                           qt[e][:, cs], start=(e == 0), stop=(e == NE - 1))
                      if bias_eng == "act":
                          nc.scalar.activation(qks[ch][:, cs], pm[:],
                                               mybir.ActivationFunctionType.Identity,
                                               bias=bqk[ch])
                      else:
                          nc.vector.tensor_scalar_add(qks[ch][:, cs], pm[:], bqk[ch])

              def emit_v(n, g):
                  """v projection (token-major) for m-tiles 4g..4g+3 of batch n."""
                  for mti in range(4 * g, 4 * g + 4):
                      mt = n * NMT + mti
                      pm = ps_m.tile([128, CS], f32, tag="m", name="pmv")
                      cs = slice(mt * 128, (mt + 1) * 128)
                      for e in range(NE):
                          nc.tensor.matmul(pm[:], qt[e][:, cs], wvt[e],
                                           start=(e == 0), stop=(e == NE - 1))
                      for h in range(2):
                          nc.vector.tensor_copy(
                              v_all[n][h][:, mti * VW:mti * VW + D],
                              pm[:, h * D:(h + 1) * D])

              ep_state = {}

              def emit_ep_pre(n, at_src):
                  """epilogue preamble: fetch this core's token block, LoRA r."""
                  ot_all = otp.tile([128, NE * LPC], bf16, tag=f"ot{n}", name="ott")
                  for e in range(NE):
                      # batch-1 runs at the tail where Act is idle: split the
                      # 8 dma_start issues (~500ns each) across SP and Act
                      # (batch-0's stay on SP: its data arrives mid-batch-1
                      # where Act is saturated with exps)
                      eng = nc.scalar if (n == 1 and e % 2) else nc.sync
                      eng.dma_start(ot_all[:, e * LPC:(e + 1) * LPC],
                                    a2a_out[n].ap()[e * 128:(e + 1) * 128, :])
                  ot = [ot_all[:, e * LPC:(e + 1) * LPC] for e in range(NE)]
                  rt_ps = ps_m.tile([128, LPC], f32, tag="m", name="rtps")
                  for e in range(NE):
                      nc.tensor.matmul(rt_ps[0:R, :], at_src[e], ot[e],
                                       start=(e == 0), stop=(e == NE - 1))
                  rt_sb = smp.tile([R, LPC], bf16, tag="rt", name="rtsb")
                  nc.vector.tensor_copy(rt_sb[:], rt_ps[0:R, :])
                  ep_state[n] = (ot, rt_sb)

              def emit_ep_eo(n, eo):
                  """epilogue: output row-block eo for batch-n token slice."""
                  ot, rt_sb = ep_state[n]
                  f_ps = ps_m.tile([128, LPC], f32, tag="m", name="fps")
                  eos = slice(eo * 128, (eo + 1) * 128)
                  for e in range(NE):
                      nc.tensor.matmul(f_ps[:], woutt_all[:, e * E + eo * 128: e * E + (eo + 1) * 128], ot[e],
                                       start=(e == 0), stop=False)
                  nc.tensor.matmul(f_ps[:], btf[:, eos], rt_sb[:],
                                   start=False, stop=True)
                  # bf16 writeback: halves the output DMA bytes (the last
                  # transfer gates the end-of-program barrier); adds ~0.11%
                  # quantization noise vs the 2e-2 gate
                  ob = obp.tile([128, LPC], bf16, tag="ob", name="obt")
                  nc.vector.tensor_scalar_add(ob[:], f_ps[:], bout[eo])
                  nc.sync.dma_start(outp_d.ap()[eos, n * LPC:(n + 1) * LPC], ob[:])

              fillers = deque()

              def pop_filler(k=1):
                  for _ in range(k):
                      if fillers:
                          fillers.popleft()()

              # ---- interleaved schedule ----
              emit_qk(0, 0)
              emit_v(0, 0)
              for n in range(N):
                  base = n * L
                  for lb in range(NLB):
                      ls = slice(base + lb * LB, base + (lb + 1) * LB)
                      lsl = slice(lb * LB, (lb + 1) * LB)
                      o_ps = None
                      for j in range(NJ):
                          if n == 0 and lb == 0 and j in (2, 4, 6):
                              emit_qk(0, j // 2)
                          if j == 0:
                              o_ps = [ps_o.tile([VW, LB], f32, tag=f"acc{h}",
                                                name=f"ops{h}") for h in range(2)]
                          # S^T pair: heads row-tiled, two m-tiles per bank-pair
                          s_ps = [ps_s.tile([128, 2 * LB], f32, tag="s", name="sps")
                                  for _ in range(2)]
                          for t in range(2):
                              ms = slice(base + (2 * j + t) * 128,
                                         base + (2 * j + t + 1) * 128)
                              for h in range(2):
                                  d0 = h * D
                                  nc.tensor.matmul(s_ps[h][:, t * LB:(t + 1) * LB],
                                                   qks[1][d0:d0 + D, ms],
                                                   qks[0][d0:d0 + D, ls],
                                                   start=True, stop=True)
                          p_t = []
                          for h in range(2):
                              pt = pp.tile([128, 2 * LB], bf16, tag="p", name="pt")
                              nc.scalar.activation(pt[:], s_ps[h][:],
                                                   mybir.ActivationFunctionType.Exp)
                              p_t.append(pt)
                          # PE filler under the exp latency
                          if n == 0 and lb == 0:
                              if j in (2, 4, 6):
                                  emit_v(0, j // 2)
                          elif n == 0 and lb < 3:
                              # batch-1 k/v0/q0 over lb 1..2 so their DVE
                              # copies retire well before batch-1 attention
                              if ((lb - 1) * NJ + j) % 2 == 0:
                                  pop_filler(1)
                          elif n == 1 and lb == 0 and j in (1, 3, 5):
                              # batch-1 v blocks just-in-time
                              emit_v(1, (j + 1) // 2)
                          elif n == 1 and lb == 0 and j in (0, 2):
                              # late batch-1 k blocks (k(1,t) is consumed from
                              # S at j=2t, so t=2,3 fit inside lb0)
                              emit_qk(1, j // 2 + 2, chs=(1,), bias_eng="dve")
                          elif n == 1 and lb < 3 and j == 6:
                              # next l-block's batch-1 q projection
                              emit_qk(1, lb + 1, chs=(0,), bias_eng="dve")
                          # P@V: per-head PSUM banks; 65th stationary column
                          # of ones accumulates the softmax denominator.
                          for t in range(2):
                              mti = 2 * j + t
                              for h in range(2):
                                  nc.tensor.matmul(o_ps[h][:, :],
                                                   v_all[n][h][:, mti * VW:(mti + 1) * VW],
                                                   p_t[h][:, t * LB:(t + 1) * LB],
                                                   start=(j == 0 and t == 0),
                                                   stop=(j == NJ - 1 and t == 1))
                      # normalization: PSUM partitions D..2D hold the
                      # replicated denominator; reciprocal + multiply on DVE
                      for h in range(2):
                          rcb = smp.tile([D, LB], f32, tag=f"rc{h}", name=f"rc{h}")
                          nc.vector.reciprocal(rcb[:], o_ps[h][D:2 * D, :])
                          nc.vector.tensor_mul(osb[n][h * D:(h + 1) * D, lsl],
                                               o_ps[h][0:D, :], rcb[:])
                      # ship the two 256-token peer blocks of this l-block
                      for jj in (2 * lb, 2 * lb + 1):
                          (nc.gpsimd.dma_start if K_GPSHIP else nc.sync.dma_start)(
                              a2a_in[n].ap()[CS * jj:CS * (jj + 1), :],
                              osb[n][:, LPC * jj:LPC * (jj + 1)])
                      # enqueue the batch-1 projections that batch-1's own
                      # attention cannot produce just-in-time: all k blocks,
                      # plus the first q and v blocks
                      if n == 0 and lb == 0:
                          for t in range(2):
                              fillers.append(lambda t=t: emit_qk(1, t, chs=(1,)))
                          fillers.append(lambda: emit_v(1, 0))
                          fillers.append(lambda: emit_qk(1, 0, chs=(0,)))
                  # end of batch: drain pending projections, launch exchange
                  if n == 0:
                      pop_filler(len(fillers))
                  if n == 1:
                      # stage the LoRA A stationaries through a tile that
                      # (x0) depends on batch-1 lb3 output: the list scheduler
                      # mis-times collectives, and an ungated epilogue
                      # Ldweights would hoist ahead of the batch-0 AllToAll
                      # and lock the PE array. Gating on lb3 also pins the
                      # whole batch-0 epilogue to the tail, where it fills
                      # the exposed batch-1 AllToAll window instead of
                      # batch-1's harmless Act-bound bubbles.
                      gate = osb[1][:, NLB * LB - NE * R:NLB * LB]
                      nc.vector.scalar_tensor_tensor(
                          at_s[:], gate, 0.0, at_all[:],
                          mybir.AluOpType.mult, mybir.AluOpType.add)
                      emit_ep_pre(0, at_gated)
                  nc.gpsimd.collective_compute(
                      "AllToAll", mybir.AluOpType.bypass,
                      ins=[a2a_in[n].ap()], outs=[a2a_out[n].ap()],
                      replica_groups=[list(range(NCORES))],
                  )
                  if n == 0:
                      for eo in range(NE):
                          fillers.append(lambda eo=eo: emit_ep_eo(0, eo))
              # tail: leftover epilogue-0 chunks overlap the batch-1 AllToAll,
              # then the batch-1 epilogue
              pop_filler(len(fillers))
              emit_ep_pre(1, at_plain)
              for eo in range(NE):
                  emit_ep_eo(1, eo)

    nc.compile()
    return nc


def _host_prep(inputs):
    q = np.asarray(inputs["query"], np.float32)
    W = np.asarray(inputs["in_proj_weight"], np.float32)
    b = np.asarray(inputs["in_proj_bias"], np.float32)
    Wout = np.asarray(inputs["out_proj_weight"], np.float32)
    bout = np.asarray(inputs["out_proj_bias"], np.float32)
    A = np.asarray(inputs["lora_A"], np.float32)
    B = np.asarray(inputs["lora_B"], np.float32)

    def e_inter(x):
        """(E, C) -> (128, NE*C): row e*128+p -> partition p, col block e."""
        C = x.shape[1]
        return np.ascontiguousarray(
            x.reshape(NE, 128, C).transpose(1, 0, 2).reshape(128, NE * C))

    qT = np.ascontiguousarray(q.transpose(2, 1, 0).reshape(E, T)).astype(BF)
    bv = b[2 * E:3 * E]
    bout_eff = bout + Wout @ bv + LORA_SCALING * (B @ (A @ bv))
    atF = e_inter(np.ascontiguousarray(A.T).astype(BF))           # (128, NE*R)
    wouttF = e_inter(np.ascontiguousarray(Wout.T).astype(BF))     # (128, NE*E)
    btfF = np.ascontiguousarray((B * LORA_SCALING).T).astype(BF)  # (R, E)
    boutF = e_inter(np.ascontiguousarray(bout_eff[:, None], np.float32))

    in_maps = []
    for c in range(NCORES):
        hs = slice(CS * c, CS * (c + 1))
        wq = W[hs, :] * SCALE
        wk = W[E + CS * c:E + CS * (c + 1), :]
        wv = W[2 * E + CS * c:2 * E + CS * (c + 1), :]
        wqkt = e_inter(np.ascontiguousarray(
            np.concatenate([wq.T, wk.T], axis=1)).astype(BF))
        wvt = e_inter(np.ascontiguousarray(wv.T).astype(BF))
        bqk = np.stack([b[hs] * SCALE, b[E + CS * c:E + CS * (c + 1)]], axis=1)
        in_maps.append({
            "qT": qT,
            "wqkt": wqkt,
            "wvt": wvt,
            "bqk": np.ascontiguousarray(bqk, np.float32),
            "woutt": wouttF,
            "at": atF,
            "btf": btfF,
            "bout": boutF,
        })
    return in_maps


def _run(inputs, trace=False):
    if "nc" not in _CACHE:
        _CACHE["nc"] = _build_nc()
    nc = _CACHE["nc"]
    in_maps = _host_prep(inputs)
    res = run_bass_kernel_spmd(nc, in_maps, core_ids=list(range(NCORES)),
                               trace=trace)
    # core c holds all E channels for tokens {(n, l): l in [256c, 256c+256)}
    full = np.empty((E, N, L), np.float32)
    for c in range(NCORES):
        o = np.asarray(res.results[c]["outp"], np.float32)  # (E, 512) bf16 -> f32
        for n in range(N):
            full[:, n, LPC * c:LPC * (c + 1)] = o[:, n * LPC:(n + 1) * LPC]
    out = np.ascontiguousarray(full.transpose(2, 1, 0))
    return out, res


def kernel(**inputs):
    out, _ = _run(inputs, trace=False)
    return out
